# revision 17
# baseline (speedup 1.0000x reference)
"""CRF forward (log-partition) on 8 Trainium2 NeuronCores.

Bidirectional (meet-in-the-middle) scaled forward algorithm with
length-sorted asymmetric chains, data-parallel over the batch.

Math: logZ_b = ln( f^T (D_{n-1}E) ... (D_0 E) p_0 ),  D_t = diag(exp(u_t)),
E = exp(tr), p_0 = onehot(START), f = exp(tr[END,:]).  Split at m = ceil(n/2):
  forward  : p_{k+1} = e_k o (E p_k),    p_0 = onehot(START)      [m steps]
  backward : y_{s+1} = c_{s+1} o (E^T y_s),  y_0 = e_{n-1} o f,
             c_s = e_{n-1-s}; a final all-ones slot yields q = E^T y.
  logZ = ln(p_m . q_{n-m}) + n*ln(kappa).
Both directions share one block-diagonal [128,128] bf16 stationary
(rows 0:64 fwd tags, rows 64:128 bwd tags), so one device step = ONE
matmul + ONE elementwise multiply per chain.

The wall-clock is pure serial latency: each chain step is a
PE -> (sem) -> DVE -> (sem) -> PE round trip of ~527ns (matmul exec +
PSUM-write drain ~210ns, DVE mul with its 120-cycle PSUM access ~225ns,
~95ns of semaphore hops), and the recurrence is strictly sequential per
lane, so wall = 256 x 527ns + overheads.  Two 64-lane chains (one per
half of the length-sorted batch) zip 1:1 through the in-order engines;
the short chain runs only ~max-mf-of-its-lanes steps (adaptive, ~122).
Emissions exp(u - ln kappa) are precomputed on host in bf16; per-step
states go to 32-step staging tiles DMA'd out in batches (SP sequencer
costs ~565ns per DMA issue) while e-tiles prefetch one group ahead on the
otherwise-idle ACT engine's HWDGE; the host picks each lane's split-point
states and does the final dot in f64.  Lanes past their half-length get
zero emissions (multiplicative chain decays to 0, no NaNs) - no
hold/injection machinery.  bf16 weights need per-matmul LDWEIGHTS
(ldw-opt rejects bf16, and mixed f32r/bf16 matmuls are invalid), but
those ~105ns loads hide inside the semaphore wait gaps on the PE.
Measured dead ends kept out of the design: asymmetric chain widths
(in-order queues convoy both chains to the slower L), per-group width
tapering and ramp-in groups (sub-sliced operands and extra boundaries
cost more than they save).
"""

import os
import sys

import numpy as np

for _p in ("/opt/trn_rl_repo", "/root/.axon_site/_ro/trn_rl_repo"):
    if os.path.isdir(_p) and _p not in sys.path:
        sys.path.append(_p)

import contextlib

import ml_dtypes

import concourse.bacc as bacc
import concourse.bass_utils as bass_utils
import concourse.tile as tile
from concourse import mybir
from concourse.bass_utils import run_bass_kernel_spmd


@contextlib.contextmanager
def _walrus_ldw_opt():
    """No-op (kept for the test harness API)."""
    yield


T = 512
N = 64  # tags
N2 = 128  # fwd tags + bwd tags stacked on partitions
BL = 128  # batch lanes per core
NCORES = 8
FA = 64  # chain A lanes/core: the longer half of the batch
FB = BL - FA  # chain B lanes/core
START_IDX = 1
END_IDX = 2
LNK = 5.113338285898717  # mean per-step log-growth of the partition mass
GRP = 64  # timesteps per DMA/staging tile
BF16 = mybir.dt.bfloat16
F32 = mybir.dt.float32


def _build_program(steps_a: int, steps_b: int, wa: list, wb: list):
    nc = bacc.Bacc("TRN2", target_bir_lowering=False, debug=False)
    ea_d = nc.dram_tensor("ea", [N2, steps_a, FA], BF16, kind="ExternalInput")
    eb_d = nc.dram_tensor("eb", [N2, steps_b, FB], BF16, kind="ExternalInput")
    # stationary W2 and initial states fused: the first matmul then depends
    # on a single DMA semaphore (PE HW allows only one sync-wait per matmul).
    init_d = nc.dram_tensor("init", [N2, N2 + BL], BF16, kind="ExternalInput")
    oa_d = nc.dram_tensor("oa", [N2, steps_a, FA], BF16, kind="ExternalOutput")
    ob_d = nc.dram_tensor("ob", [N2, steps_b, FB], BF16, kind="ExternalOutput")

    e_drams = (ea_d, eb_d)
    o_drams = (oa_d, ob_d)
    steps = (steps_a, steps_b)
    fs = (FA, FB)

    # Group schedule: uniform 32-step groups (ramp-in prologue groups and
    # per-group width tapering both measured slower on hardware).
    bounds = []
    s = 0
    while s < max(steps_a, steps_b):
        bounds.append((s, GRP))
        s += GRP

    def glen(h, g):
        if g >= len(bounds):
            return 0
        st, ln = bounds[g]
        return max(0, min(ln, steps[h] - st))

    def fw(h, g):
        return fs[h]

    with tile.TileContext(nc) as tc:
        with (
            tc.tile_pool(name="singles", bufs=1) as singles,
            tc.tile_pool(name="ea", bufs=3) as e_pool_a,
            tc.tile_pool(name="eb", bufs=3) as e_pool_b,
            tc.tile_pool(name="sta", bufs=3) as st_pool_a,
            tc.tile_pool(name="stb", bufs=3) as st_pool_b,
            tc.tile_pool(name="za", bufs=3, space="PSUM") as z_pool_a,
            tc.tile_pool(name="zb", bufs=3, space="PSUM") as z_pool_b,
            tc.tile_pool(name="zd", bufs=2, space="PSUM") as z_pool_d,
        ):
            init_sb = singles.tile([N2, N2 + BL], BF16)
            nc.sync.dma_start(out=init_sb, in_=init_d[:, :])
            w_sb = init_sb[:, 0:N2]
            e_pools = (e_pool_a, e_pool_b)
            st_pools = (st_pool_a, st_pool_b)
            z_pools = (z_pool_a, z_pool_b)
            p_cur = [
                init_sb[:, N2 : N2 + FA],
                init_sb[:, N2 + FA : N2 + FA + FB],
            ]

            # e-tiles double-buffered and PREFETCHED one group ahead on the
            # otherwise-idle ACT engine's HWDGE so the serial chain never
            # stalls on an emission load; out-stores go through SP.
            e_q = [[], []]
            st_sbs = [None, None]

            def fetch(h, g):
                n = glen(h, g)
                if n <= 0:
                    return
                st0 = bounds[g][0]
                e_sb = e_pools[h].tile([N2, n, fs[h]], BF16, tag=f"e{h}")
                if g == 0:
                    # split the first load so the serial chain starts after
                    # a ~64KB transfer instead of the full group (~1MB)
                    nc.scalar.dma_start(
                        out=e_sb[:, 0:4, :], in_=e_drams[h][:, 0:4, :]
                    )
                    nc.scalar.dma_start(
                        out=e_sb[:, 4:n, :], in_=e_drams[h][:, 4:n, :]
                    )
                else:
                    nc.scalar.dma_start(
                        out=e_sb, in_=e_drams[h][:, st0 : st0 + n, :]
                    )
                e_q[h].append(e_sb)

            for h in range(2):
                fetch(h, 0)
            for g in range(len(bounds)):
                if glen(0, g) <= 0 and glen(1, g) <= 0:
                    break
                for h in range(2):
                    fetch(h, g + 1)
                    if glen(h, g) > 0:
                        st_sb = st_pools[h].tile(
                            [N2, glen(h, g), fs[h]], BF16, tag=f"st{h}"
                        )
                        st_sbs[h] = st_sb
                g0 = bounds[g][0]
                for k in range(bounds[g][1]):
                    for h in range(2):
                        if g0 + k >= steps[h]:
                            continue
                        z = z_pools[h].tile([N2, fs[h]], F32, tag=f"z{h}")
                        nc.tensor.matmul(z, w_sb, p_cur[h], start=True, stop=True)
                        p_new = st_sbs[h][:, k, :]
                        nc.vector.tensor_mul(p_new, z, e_q[h][0][:, k, :])
                        p_cur[h] = p_new
                        # dummy matmuls keep the PE busy through the sem-wait
                        # gaps so DVFS holds the 2.4GHz pstate (idle-gapped PE
                        # measures 1.2GHz: LDWEIGHTS 105ns = 128 x 0.833ns)
                        for _ in range(2):
                            zd = z_pool_d.tile([N2, 16], F32, tag="zd")
                            nc.tensor.matmul(
                                zd,
                                w_sb,
                                init_sb[:, 0:16],
                                start=True,
                                stop=True,
                            )
                for h in range(2):
                    if glen(h, g) > 0:
                        e_q[h].pop(0)
                        nc.sync.dma_start(
                            out=o_drams[h][:, g0 : g0 + glen(h, g), :],
                            in_=st_sbs[h],
                        )
    nc.compile()
    return nc


def _split_lengths(lens: np.ndarray):
    mf = (lens + 1) // 2  # forward steps, 1..256
    mb = lens - mf  # backward device steps (incl. final ones-slot), 0..256
    return mf, mb


def _plan(lens: np.ndarray):
    """Global length-sorted lane -> (chain, core, slot) assignment."""
    order = np.argsort(-lens, kind="stable")  # longest first
    na = NCORES * FA
    a_lanes = order[:na]  # chain A: lane a_lanes[i] -> core i%8, slot i//8
    b_lanes = order[na:]
    mf, _ = _split_lengths(lens)
    steps_a = int(mf[a_lanes].max())
    steps_b = int(mf[b_lanes].max())

    def group_widths(lanes, F, steps):
        # slot j on core c holds lanes[j*NCORES + c]; all slots with ANY
        # core still live keep the whole column range (SPMD: one width for
        # all cores).  Lanes are sorted desc so live slots are a prefix.
        # NOTE: tapering widths to the live-lane prefix measured SLOWER on
        # hardware (sub-sliced matmul/mul operands cost ~+40ns each and the
        # column-sliced DMAs go strided), so keep constant full width.
        return [F for _ in range(0, steps, GRP)]

    wa = group_widths(a_lanes, FA, steps_a)
    wb = group_widths(b_lanes, FB, steps_b)
    return a_lanes, b_lanes, steps_a, steps_b, wa, wb


def _emissions(u: np.ndarray, lens: np.ndarray, E: np.ndarray, steps: int):
    """u [L, T, N] f32, lens [L] -> e2 [L, N2, steps] f32 and y0 [L, N]."""
    f = E[END_IDX]
    ex = np.exp(u.astype(np.float64) - LNK).astype(np.float32)  # [L, T, N]
    mf, mb = _split_lengths(lens)
    d = np.arange(steps)
    fwd = np.where(
        (d[None, :, None] < mf[:, None, None]), ex[:, :steps, :], 0.0
    )  # [L, steps, N]
    idx = np.clip(lens[:, None] - 2 - d[None, :], 0, T - 1)  # [L, steps]
    bwd = np.take_along_axis(ex, idx[:, :, None], axis=1)
    bwd = np.where((d[None, :, None] <= mb[:, None, None] - 2), bwd, 0.0)
    bwd = np.where(d[None, :, None] == (mb[:, None, None] - 1), 1.0, bwd)
    e2 = np.concatenate([fwd, bwd], axis=2).transpose(0, 2, 1)  # [L, N2, steps]
    elast = np.take_along_axis(
        ex, (lens[:, None, None] - 1).astype(np.int64), axis=1
    )[:, 0, :]  # [L, N] = ex[b, n-1, :]
    y0 = (elast * f[None, :]) * (mb > 0)[:, None]
    return e2, y0


def _build_in_maps(unary: np.ndarray, tr: np.ndarray, lens: np.ndarray):
    E = np.exp(tr.astype(np.float64)).astype(np.float32)
    a_lanes, b_lanes, steps_a, steps_b, wa, wb = _plan(lens)
    w2 = np.zeros((N2, N2), dtype=np.float32)
    w2[0:N, 0:N] = E.T  # lhsT[j, i] = E[i, j]:   z_fwd = E p
    w2[N:N2, N:N2] = E  # lhsT[64+j, 64+i] = E[j, i]: z_bwd = E^T y
    plan = (a_lanes, b_lanes, steps_a, steps_b, wa, wb)

    in_maps = []
    for c in range(NCORES):
        init = np.zeros((N2, N2 + BL), dtype=np.float32)
        init[:, 0:N2] = w2
        core_map = {}
        for h, (lanes, F, steps, ename) in enumerate(
            ((a_lanes, FA, steps_a, "ea"), (b_lanes, FB, steps_b, "eb"))
        ):
            mine = lanes[c::NCORES]  # slots in order
            e2, y0 = _emissions(unary[mine], lens[mine], E, steps)
            core_map[ename] = np.ascontiguousarray(
                e2.transpose(1, 2, 0)
            ).astype(ml_dtypes.bfloat16)  # [N2, steps, F]
            col = N2 + h * FA  # chain A cols [N2, N2+FA), B [N2+FA, N2+BL)
            init[START_IDX, col : col + F] = 1.0
            init[N:N2, col : col + F] = y0.T
        core_map["init"] = init.astype(ml_dtypes.bfloat16)
        in_maps.append(core_map)
    return in_maps, plan


def _postprocess(results, tr: np.ndarray, lens: np.ndarray, plan) -> np.ndarray:
    a_lanes, b_lanes = plan[0], plan[1]
    E = np.exp(tr.astype(np.float64))
    f = E[END_IDX]
    mf, mb = _split_lengths(lens)
    out = np.empty(lens.shape[0], dtype=np.float64)
    for lanes, oname in ((a_lanes, "oa"), (b_lanes, "ob")):
        for c in range(NCORES):
            res = np.asarray(results[c][oname]).astype(np.float64)  # [N2,S,F]
            mine = lanes[c::NCORES]
            for slot, gb in enumerate(mine):
                p = res[0:N, mf[gb] - 1, slot]
                q = res[N:N2, mb[gb] - 1, slot] if mb[gb] >= 1 else f
                out[gb] = np.log(np.dot(p, q)) + lens[gb] * LNK
    return out.astype(np.float32)


def kernel(unary: np.ndarray, trans: np.ndarray, lengths: np.ndarray) -> np.ndarray:
    unary = np.asarray(unary, dtype=np.float32)  # [B, T, N]
    tr = np.asarray(trans, dtype=np.float32)[0]  # [N, N]
    lens = np.asarray(lengths).astype(np.int64)  # [B]
    B = unary.shape[0]
    assert unary.shape == (B, T, N) and B == NCORES * BL

    in_maps, plan = _build_in_maps(unary, tr, lens)
    nc = _build_program(plan[2], plan[3], plan[4], plan[5])
    res = run_bass_kernel_spmd(nc, in_maps, list(range(NCORES)))
    return _postprocess(res.results, tr, lens, plan)


# revision 18
# speedup vs baseline: 1.2318x; 1.2318x over previous
"""CRF forward (log-partition) on 8 Trainium2 NeuronCores.

Bidirectional (meet-in-the-middle) scaled forward algorithm with
length-sorted asymmetric chains, data-parallel over the batch.

Math: logZ_b = ln( f^T (D_{n-1}E) ... (D_0 E) p_0 ),  D_t = diag(exp(u_t)),
E = exp(tr), p_0 = onehot(START), f = exp(tr[END,:]).  Split at m = ceil(n/2):
  forward  : p_{k+1} = e_k o (E p_k),    p_0 = onehot(START)      [m steps]
  backward : y_{s+1} = c_{s+1} o (E^T y_s),  y_0 = e_{n-1} o f,
             c_s = e_{n-1-s}; a final all-ones slot yields q = E^T y.
  logZ = ln(p_m . q_{n-m}) + n*ln(kappa).
Both directions share one block-diagonal [128,128] bf16 stationary
(rows 0:64 fwd tags, rows 64:128 bwd tags), so one device step = ONE
matmul + ONE elementwise multiply per chain.

The wall-clock is pure serial latency: each chain step is a
PE -> (sem) -> DVE -> (sem) -> PE round trip of ~527ns (matmul exec +
PSUM-write drain ~210ns, DVE mul with its 120-cycle PSUM access ~225ns,
~95ns of semaphore hops), and the recurrence is strictly sequential per
lane, so wall = 256 x 527ns + overheads.  Two 64-lane chains (one per
half of the length-sorted batch) zip 1:1 through the in-order engines;
the short chain runs only ~max-mf-of-its-lanes steps (adaptive, ~122).
Emissions exp(u - ln kappa) are precomputed on host in bf16; per-step
states go to 32-step staging tiles DMA'd out in batches (SP sequencer
costs ~565ns per DMA issue) while e-tiles prefetch one group ahead on the
otherwise-idle ACT engine's HWDGE; the host picks each lane's split-point
states and does the final dot in f64.  Lanes past their half-length get
zero emissions (multiplicative chain decays to 0, no NaNs) - no
hold/injection machinery.  bf16 weights need per-matmul LDWEIGHTS
(ldw-opt rejects bf16, and mixed f32r/bf16 matmuls are invalid), but
those ~105ns loads hide inside the semaphore wait gaps on the PE.
Measured dead ends kept out of the design: asymmetric chain widths
(in-order queues convoy both chains to the slower L), per-group width
tapering and ramp-in groups (sub-sliced operands and extra boundaries
cost more than they save).
"""

import os
import sys

import numpy as np

for _p in ("/opt/trn_rl_repo", "/root/.axon_site/_ro/trn_rl_repo"):
    if os.path.isdir(_p) and _p not in sys.path:
        sys.path.append(_p)

import contextlib

import ml_dtypes

import concourse.bacc as bacc
import concourse.bass_utils as bass_utils
import concourse.tile as tile
from concourse import mybir
from concourse.bass_utils import run_bass_kernel_spmd


@contextlib.contextmanager
def _walrus_ldw_opt():
    """No-op (kept for the test harness API)."""
    yield


T = 512
N = 64  # tags
N2 = 128  # fwd tags + bwd tags stacked on partitions
BL = 128  # batch lanes per core
NCORES = 8
FA = 64  # chain A lanes/core: the longer half of the batch
FB = BL - FA  # chain B lanes/core
START_IDX = 1
END_IDX = 2
LNK = 5.113338285898717  # mean per-step log-growth of the partition mass
GRP = 64  # timesteps per DMA/staging tile
BF16 = mybir.dt.bfloat16
F32 = mybir.dt.float32


def _build_program(steps_a: int, steps_b: int, wa: list, wb: list):
    nc = bacc.Bacc("TRN2", target_bir_lowering=False, debug=False)
    ea_d = nc.dram_tensor("ea", [N2, steps_a, FA], BF16, kind="ExternalInput")
    eb_d = nc.dram_tensor("eb", [N2, steps_b, FB], BF16, kind="ExternalInput")
    # stationary W2 and initial states fused: the first matmul then depends
    # on a single DMA semaphore (PE HW allows only one sync-wait per matmul).
    init_d = nc.dram_tensor("init", [N2, N2 + BL], BF16, kind="ExternalInput")
    oa_d = nc.dram_tensor("oa", [N2, steps_a, FA], BF16, kind="ExternalOutput")
    ob_d = nc.dram_tensor("ob", [N2, steps_b, FB], BF16, kind="ExternalOutput")

    e_drams = (ea_d, eb_d)
    o_drams = (oa_d, ob_d)
    steps = (steps_a, steps_b)
    fs = (FA, FB)

    # Group schedule: uniform 32-step groups (ramp-in prologue groups and
    # per-group width tapering both measured slower on hardware).
    bounds = []
    s = 0
    while s < max(steps_a, steps_b):
        bounds.append((s, GRP))
        s += GRP

    def glen(h, g):
        if g >= len(bounds):
            return 0
        st, ln = bounds[g]
        return max(0, min(ln, steps[h] - st))

    def fw(h, g):
        return fs[h]

    with tile.TileContext(nc) as tc:
        with (
            tc.tile_pool(name="singles", bufs=1) as singles,
            tc.tile_pool(name="ea", bufs=3) as e_pool_a,
            tc.tile_pool(name="eb", bufs=3) as e_pool_b,
            tc.tile_pool(name="sta", bufs=3) as st_pool_a,
            tc.tile_pool(name="stb", bufs=3) as st_pool_b,
            tc.tile_pool(name="za", bufs=4, space="PSUM") as z_pool_a,
            tc.tile_pool(name="zb", bufs=4, space="PSUM") as z_pool_b,
        ):
            init_sb = singles.tile([N2, N2 + BL], BF16)
            nc.sync.dma_start(out=init_sb, in_=init_d[:, :])
            w_sb = init_sb[:, 0:N2]
            e_pools = (e_pool_a, e_pool_b)
            st_pools = (st_pool_a, st_pool_b)
            z_pools = (z_pool_a, z_pool_b)
            p_cur = [
                init_sb[:, N2 : N2 + FA],
                init_sb[:, N2 + FA : N2 + FA + FB],
            ]

            # e-tiles double-buffered and PREFETCHED one group ahead on the
            # otherwise-idle ACT engine's HWDGE so the serial chain never
            # stalls on an emission load; out-stores go through SP.
            e_q = [[], []]
            st_sbs = [None, None]

            def fetch(h, g):
                n = glen(h, g)
                if n <= 0:
                    return
                st0 = bounds[g][0]
                e_sb = e_pools[h].tile([N2, n, fs[h]], BF16, tag=f"e{h}")
                if g == 0:
                    # split the first load so the serial chain starts after
                    # a ~64KB transfer instead of the full group (~1MB)
                    nc.scalar.dma_start(
                        out=e_sb[:, 0:4, :], in_=e_drams[h][:, 0:4, :]
                    )
                    nc.scalar.dma_start(
                        out=e_sb[:, 4:n, :], in_=e_drams[h][:, 4:n, :]
                    )
                else:
                    nc.scalar.dma_start(
                        out=e_sb, in_=e_drams[h][:, st0 : st0 + n, :]
                    )
                e_q[h].append(e_sb)

            for h in range(2):
                fetch(h, 0)
            for g in range(len(bounds)):
                if glen(0, g) <= 0 and glen(1, g) <= 0:
                    break
                for h in range(2):
                    fetch(h, g + 1)
                    if glen(h, g) > 0:
                        st_sb = st_pools[h].tile(
                            [N2, glen(h, g), fs[h]], BF16, tag=f"st{h}"
                        )
                        st_sbs[h] = st_sb
                g0 = bounds[g][0]
                for k in range(bounds[g][1]):
                    for h in range(2):
                        if g0 + k >= steps[h]:
                            continue
                        z = z_pools[h].tile([N2, fs[h]], F32, tag=f"z{h}")
                        nc.tensor.matmul(z, w_sb, p_cur[h], start=True, stop=True)
                        p_new = st_sbs[h][:, k, :]
                        nc.vector.tensor_mul(p_new, z, e_q[h][0][:, k, :])
                        p_cur[h] = p_new
                for h in range(2):
                    if glen(h, g) > 0:
                        e_q[h].pop(0)
                        nc.sync.dma_start(
                            out=o_drams[h][:, g0 : g0 + glen(h, g), :],
                            in_=st_sbs[h],
                        )
    nc.compile()
    return nc


def _split_lengths(lens: np.ndarray):
    mf = (lens + 1) // 2  # forward steps, 1..256
    mb = lens - mf  # backward device steps (incl. final ones-slot), 0..256
    return mf, mb


def _plan(lens: np.ndarray):
    """Global length-sorted lane -> (chain, core, slot) assignment."""
    order = np.argsort(-lens, kind="stable")  # longest first
    na = NCORES * FA
    a_lanes = order[:na]  # chain A: lane a_lanes[i] -> core i%8, slot i//8
    b_lanes = order[na:]
    mf, _ = _split_lengths(lens)
    steps_a = int(mf[a_lanes].max())
    steps_b = int(mf[b_lanes].max())

    def group_widths(lanes, F, steps):
        # slot j on core c holds lanes[j*NCORES + c]; all slots with ANY
        # core still live keep the whole column range (SPMD: one width for
        # all cores).  Lanes are sorted desc so live slots are a prefix.
        # NOTE: tapering widths to the live-lane prefix measured SLOWER on
        # hardware (sub-sliced matmul/mul operands cost ~+40ns each and the
        # column-sliced DMAs go strided), so keep constant full width.
        return [F for _ in range(0, steps, GRP)]

    wa = group_widths(a_lanes, FA, steps_a)
    wb = group_widths(b_lanes, FB, steps_b)
    return a_lanes, b_lanes, steps_a, steps_b, wa, wb


def _emissions(u: np.ndarray, lens: np.ndarray, E: np.ndarray, steps: int):
    """u [L, T, N] f32, lens [L] -> e2 [L, N2, steps] f32 and y0 [L, N]."""
    f = E[END_IDX]
    ex = np.exp(u.astype(np.float64) - LNK).astype(np.float32)  # [L, T, N]
    mf, mb = _split_lengths(lens)
    d = np.arange(steps)
    fwd = np.where(
        (d[None, :, None] < mf[:, None, None]), ex[:, :steps, :], 0.0
    )  # [L, steps, N]
    idx = np.clip(lens[:, None] - 2 - d[None, :], 0, T - 1)  # [L, steps]
    bwd = np.take_along_axis(ex, idx[:, :, None], axis=1)
    bwd = np.where((d[None, :, None] <= mb[:, None, None] - 2), bwd, 0.0)
    bwd = np.where(d[None, :, None] == (mb[:, None, None] - 1), 1.0, bwd)
    e2 = np.concatenate([fwd, bwd], axis=2).transpose(0, 2, 1)  # [L, N2, steps]
    elast = np.take_along_axis(
        ex, (lens[:, None, None] - 1).astype(np.int64), axis=1
    )[:, 0, :]  # [L, N] = ex[b, n-1, :]
    y0 = (elast * f[None, :]) * (mb > 0)[:, None]
    return e2, y0


def _build_in_maps(unary: np.ndarray, tr: np.ndarray, lens: np.ndarray):
    E = np.exp(tr.astype(np.float64)).astype(np.float32)
    a_lanes, b_lanes, steps_a, steps_b, wa, wb = _plan(lens)
    w2 = np.zeros((N2, N2), dtype=np.float32)
    w2[0:N, 0:N] = E.T  # lhsT[j, i] = E[i, j]:   z_fwd = E p
    w2[N:N2, N:N2] = E  # lhsT[64+j, 64+i] = E[j, i]: z_bwd = E^T y
    plan = (a_lanes, b_lanes, steps_a, steps_b, wa, wb)

    in_maps = []
    for c in range(NCORES):
        init = np.zeros((N2, N2 + BL), dtype=np.float32)
        init[:, 0:N2] = w2
        core_map = {}
        for h, (lanes, F, steps, ename) in enumerate(
            ((a_lanes, FA, steps_a, "ea"), (b_lanes, FB, steps_b, "eb"))
        ):
            mine = lanes[c::NCORES]  # slots in order
            e2, y0 = _emissions(unary[mine], lens[mine], E, steps)
            core_map[ename] = np.ascontiguousarray(
                e2.transpose(1, 2, 0)
            ).astype(ml_dtypes.bfloat16)  # [N2, steps, F]
            col = N2 + h * FA  # chain A cols [N2, N2+FA), B [N2+FA, N2+BL)
            init[START_IDX, col : col + F] = 1.0
            init[N:N2, col : col + F] = y0.T
        core_map["init"] = init.astype(ml_dtypes.bfloat16)
        in_maps.append(core_map)
    return in_maps, plan


def _postprocess(results, tr: np.ndarray, lens: np.ndarray, plan) -> np.ndarray:
    a_lanes, b_lanes = plan[0], plan[1]
    E = np.exp(tr.astype(np.float64))
    f = E[END_IDX]
    mf, mb = _split_lengths(lens)
    out = np.empty(lens.shape[0], dtype=np.float64)
    for lanes, oname in ((a_lanes, "oa"), (b_lanes, "ob")):
        for c in range(NCORES):
            res = np.asarray(results[c][oname]).astype(np.float64)  # [N2,S,F]
            mine = lanes[c::NCORES]
            for slot, gb in enumerate(mine):
                p = res[0:N, mf[gb] - 1, slot]
                q = res[N:N2, mb[gb] - 1, slot] if mb[gb] >= 1 else f
                out[gb] = np.log(np.dot(p, q)) + lens[gb] * LNK
    return out.astype(np.float32)


def kernel(unary: np.ndarray, trans: np.ndarray, lengths: np.ndarray) -> np.ndarray:
    unary = np.asarray(unary, dtype=np.float32)  # [B, T, N]
    tr = np.asarray(trans, dtype=np.float32)[0]  # [N, N]
    lens = np.asarray(lengths).astype(np.int64)  # [B]
    B = unary.shape[0]
    assert unary.shape == (B, T, N) and B == NCORES * BL

    in_maps, plan = _build_in_maps(unary, tr, lens)
    nc = _build_program(plan[2], plan[3], plan[4], plan[5])
    res = run_bass_kernel_spmd(nc, in_maps, list(range(NCORES)))
    return _postprocess(res.results, tr, lens, plan)
